# revision 2
# baseline (speedup 1.0000x reference)
"""Triu-scatter kernel for Trainium2 (8 NeuronCores).

Reference op: out[b] = scatter of packed upper-triangle vector (524800) into a
(1024, 1024) matrix, zeros elsewhere.  Row r of each output matrix is r zeros
followed by a contiguous slice of the packed input (length 1024-r), so the
whole op is pure structured data movement.

Distribution: output rows are interleaved across cores (core j owns rows
r = j mod 8) with the full batch of 128 kept per core so DMAs use all 128
partitions.  Row lengths per core differ only by j (<8 elements), so after
padding each row slice (leading zeros), one SPMD NEFF serves all cores.

Per core the device does:
  - data: DRAM->DRAM copies, one per group of G rows, each a 3D affine access
    pattern [batch=128][row-in-group=G][contiguous run]
  - zeros for cols [0, 8*m0): SBUF zero tile -> DRAM, same 3D structure
The host packs each core's input so that the leading pad of each row slice is
zeros, which lands exactly on the output cols between 8*m0 and the row start.

Variants (KERNEL_VARIANT env, default "full"):
  full - kernel writes every output element (data + zeros).
  noz  - kernel writes only data rows; relies on run_bass_kernel_spmd's
         documented contract that ExternalOutput buffers are pre-zeroed
         (native path: out_maps = np.zeros; axon path: donated zero buffers).
"""

import os

import numpy as np

MAT = 1024
NCORES = 8
MPC = MAT // NCORES  # kernel rows per core = 128
B = 128              # full batch per core

VARIANT = os.environ.get("KERNEL_VARIANT", "full")
G = int(os.environ.get("KERNEL_G", "4" if VARIANT == "full" else "1"))

_ROW_START = [r * MAT - r * (r - 1) // 2 for r in range(MAT)]


def _schedule():
    groups = []
    m0 = 0
    while m0 < MPC:
        g = min(G, MPC - m0)
        groups.append((m0, g))
        m0 += g
    return groups


def _padded_len(groups):
    return sum(g * (MAT - 8 * m0) for m0, g in groups)


def _build_nc(groups, P, write_zeros):
    import concourse.bass as bass
    from concourse import mybir

    nc = bass.Bass()
    X = nc.dram_tensor("inputs", [B, P], mybir.dt.float32, kind="ExternalInput")
    Y = nc.dram_tensor("out", [B, MPC, MAT], mybir.dt.float32, kind="ExternalOutput")

    data_aps = []
    zero_aps = []
    off = 0
    for m0, g in groups:
        L = MAT - 8 * m0
        src = bass.AP(X, off, [[P, B], [L, g], [1, L]])
        dst = bass.AP(Y, m0 * MAT + 8 * m0, [[MPC * MAT, B], [MAT, g], [1, L]])
        data_aps.append((dst, src))
        if m0 > 0 and write_zeros:
            zdst = bass.AP(Y, m0 * MAT, [[MPC * MAT, B], [MAT, g], [1, 8 * m0]])
            zero_aps.append((zdst, 8 * m0 * g))
        off += g * L

    if write_zeros:
        zcols = max((n for _, n in zero_aps), default=1)
        with (
            nc.sbuf_tensor([128, zcols], mybir.dt.float32) as zt,
            nc.semaphore("zsem") as zsem,
            nc.semaphore("ssem") as ssem,
            nc.semaphore("asem") as asem,
            nc.Block() as block,
        ):

            @block.vector
            def _(vector):
                vector.memset(zt[:], 0).then_inc(zsem, 1)

            @block.sync
            def _(sync):
                n = 0
                for dst, src in data_aps:
                    sync.dma_start(out=dst, in_=src).then_inc(ssem, 16)
                    n += 16
                sync.wait_ge(ssem, n)

            @block.scalar
            def _(scalar):
                scalar.wait_ge(zsem, 1)
                n = 0
                for zdst, ncols in zero_aps:
                    scalar.dma_start(out=zdst, in_=zt[:, :ncols]).then_inc(asem, 16)
                    n += 16
                scalar.wait_ge(asem, n)
    else:
        # data only; split the DMAs across both HWDGE rings (sync + scalar)
        with (
            nc.semaphore("ssem") as ssem,
            nc.semaphore("asem") as asem,
            nc.Block() as block,
        ):
            sync_dmas = data_aps[0::2]
            scalar_dmas = data_aps[1::2]

            @block.sync
            def _(sync):
                n = 0
                for dst, src in sync_dmas:
                    sync.dma_start(out=dst, in_=src).then_inc(ssem, 16)
                    n += 16
                sync.wait_ge(ssem, n)

            @block.scalar
            def _(scalar):
                n = 0
                for dst, src in scalar_dmas:
                    scalar.dma_start(out=dst, in_=src).then_inc(asem, 16)
                    n += 16
                scalar.wait_ge(asem, n)

    return nc


def _pack_core_inputs(x, groups, P):
    """Build the per-core padded input buffers (core j gets rows r = j mod 8)."""
    in_maps = []
    for j in range(NCORES):
        xc = np.zeros((B, P), dtype=np.float32)
        off = 0
        for m0, g in groups:
            L = MAT - 8 * m0
            for gg in range(g):
                r = 8 * (m0 + gg) + j
                a = MAT - r              # actual data length for this row
                z = L - a                # leading zeros (= 8*gg + j)
                s = _ROW_START[r]
                xc[:, off + z : off + L] = x[:, s : s + a]
                off += L
        in_maps.append({"inputs": xc})
    return in_maps


def run(inputs, trace=False):
    from concourse.bass_utils import run_bass_kernel_spmd

    x = np.ascontiguousarray(np.asarray(inputs), dtype=np.float32)
    assert x.shape == (B, MAT * (MAT + 1) // 2), x.shape

    groups = _schedule()
    P = _padded_len(groups)
    in_maps = _pack_core_inputs(x, groups, P)

    nc = _build_nc(groups, P, write_zeros=(VARIANT == "full"))
    res = run_bass_kernel_spmd(
        nc, in_maps, core_ids=list(range(NCORES)), trace=trace
    )

    out = np.empty((B, MAT, MAT), dtype=np.float32)
    for j in range(NCORES):
        out[:, j::8, :] = res.results[j]["out"]
    return out, res


def kernel(inputs):
    out, _ = run(inputs, trace=False)
    return out


# revision 4
# speedup vs baseline: 1.3059x; 1.3059x over previous
"""Triu-scatter kernel for Trainium2 (8 NeuronCores).

Reference op: out[b] = scatter of packed upper-triangle vector (524800) into a
(1024, 1024) matrix, zeros elsewhere.  Row r of each output matrix is r zeros
followed by a contiguous slice of the packed input (length 1024-r), so the
whole op is pure structured data movement.

Distribution: output rows are interleaved across cores (core j owns rows
r = j mod 8) with the full batch of 128 kept per core so DMAs use all 128
partitions.  Row lengths per core differ only by j (<8 elements), so after
padding each row slice (leading zeros), one SPMD NEFF serves all cores.

Per core the device does:
  - data: DRAM->DRAM copies, one per group of G rows, each a 3D affine access
    pattern [batch=128][row-in-group=G][contiguous run]
  - zeros for cols [0, 8*m0): SBUF zero tile -> DRAM, same 3D structure
The host packs each core's input so that the leading pad of each row slice is
zeros, which lands exactly on the output cols between 8*m0 and the row start.

Variants (KERNEL_VARIANT env, default "full"):
  full - kernel writes every output element (data + zeros).
  noz  - kernel writes only data rows; relies on run_bass_kernel_spmd's
         documented contract that ExternalOutput buffers are pre-zeroed
         (native path: out_maps = np.zeros; axon path: donated zero buffers).
"""

import os

import numpy as np

MAT = 1024
NCORES = 8
MPC = MAT // NCORES  # kernel rows per core = 128
B = 128              # full batch per core

VARIANT = os.environ.get("KERNEL_VARIANT", "full")
G = int(os.environ.get("KERNEL_G", "4" if VARIANT == "full" else "1"))
RINGS = int(os.environ.get("KERNEL_RINGS", "2"))

_ROW_START = [r * MAT - r * (r - 1) // 2 for r in range(MAT)]


def _schedule():
    groups = []
    m0 = 0
    while m0 < MPC:
        g = min(G, MPC - m0)
        groups.append((m0, g))
        m0 += g
    return groups


def _padded_len(groups):
    return sum(g * (MAT - 8 * m0) for m0, g in groups)


def _build_nc(groups, P, write_zeros):
    import concourse.bass as bass
    from concourse import mybir

    nc = bass.Bass()
    X = nc.dram_tensor("inputs", [B, P], mybir.dt.float32, kind="ExternalInput")
    Y = nc.dram_tensor("out", [B, MPC, MAT], mybir.dt.float32, kind="ExternalOutput")

    data_aps = []
    zero_aps = []
    off = 0
    for m0, g in groups:
        L = MAT - 8 * m0
        src = bass.AP(X, off, [[P, B], [L, g], [1, L]])
        dst = bass.AP(Y, m0 * MAT + 8 * m0, [[MPC * MAT, B], [MAT, g], [1, L]])
        data_aps.append((dst, src))
        if m0 > 0 and write_zeros:
            zdst = bass.AP(Y, m0 * MAT, [[MPC * MAT, B], [MAT, g], [1, 8 * m0]])
            zero_aps.append((zdst, 8 * m0 * g))
        off += g * L

    if write_zeros:
        zcols = max((n for _, n in zero_aps), default=1)
        with (
            nc.sbuf_tensor([128, zcols], mybir.dt.float32) as zt,
            nc.semaphore("zsem") as zsem,
            nc.semaphore("ssem") as ssem,
            nc.semaphore("asem") as asem,
            nc.Block() as block,
        ):

            @block.vector
            def _(vector):
                vector.memset(zt[:], 0).then_inc(zsem, 1)

            @block.sync
            def _(sync):
                n = 0
                for dst, src in data_aps:
                    sync.dma_start(out=dst, in_=src).then_inc(ssem, 16)
                    n += 16
                sync.wait_ge(ssem, n)

            @block.scalar
            def _(scalar):
                scalar.wait_ge(zsem, 1)
                n = 0
                for zdst, ncols in zero_aps:
                    scalar.dma_start(out=zdst, in_=zt[:, :ncols]).then_inc(asem, 16)
                    n += 16
                scalar.wait_ge(asem, n)
    else:
        # data only; split the DMAs round-robin across the issuing rings
        from contextlib import ExitStack

        names = ["sync", "scalar", "gpsimd"][:RINGS]
        streams = {n: [] for n in names}
        for i, pair in enumerate(data_aps):
            streams[names[i % len(names)]].append(pair)

        def make_fn(pairs, sem):
            def fn(eng):
                n = 0
                for dst, src in pairs:
                    eng.dma_start(out=dst, in_=src).then_inc(sem, 16)
                    n += 16
                eng.wait_ge(sem, n)

            return fn

        with ExitStack() as stack:
            sems = {n: stack.enter_context(nc.semaphore(f"sem_{n}")) for n in names}
            block = stack.enter_context(nc.Block())
            for n in names:
                getattr(block, n)(make_fn(streams[n], sems[n]))

    return nc


def _pack_core_inputs(x, groups, P):
    """Build the per-core padded input buffers (core j gets rows r = j mod 8)."""
    in_maps = []
    for j in range(NCORES):
        xc = np.zeros((B, P), dtype=np.float32)
        off = 0
        for m0, g in groups:
            L = MAT - 8 * m0
            for gg in range(g):
                r = 8 * (m0 + gg) + j
                a = MAT - r              # actual data length for this row
                z = L - a                # leading zeros (= 8*gg + j)
                s = _ROW_START[r]
                xc[:, off + z : off + L] = x[:, s : s + a]
                off += L
        in_maps.append({"inputs": xc})
    return in_maps


def run(inputs, trace=False):
    from concourse.bass_utils import run_bass_kernel_spmd

    x = np.ascontiguousarray(np.asarray(inputs), dtype=np.float32)
    assert x.shape == (B, MAT * (MAT + 1) // 2), x.shape

    groups = _schedule()
    P = _padded_len(groups)
    in_maps = _pack_core_inputs(x, groups, P)

    nc = _build_nc(groups, P, write_zeros=(VARIANT == "full"))
    res = run_bass_kernel_spmd(
        nc, in_maps, core_ids=list(range(NCORES)), trace=trace
    )

    out = np.empty((B, MAT, MAT), dtype=np.float32)
    for j in range(NCORES):
        out[:, j::8, :] = res.results[j]["out"]
    return out, res


def kernel(inputs):
    out, _ = run(inputs, trace=False)
    return out
